# revision 30
# baseline (speedup 1.0000x reference)
"""Causal attention kernel for Trainium2 (Bass/Tile), data-parallel over batch.

Problem (hardcoded): x[64,512,1024] f32, Wq/Wk/Wv[1024,256], bq/bk/bv[256].
  q = x@Wq+bq ; k = x@Wk+bk ; v = x@Wv+bv
  out = softmax(causal(q k^T / sqrt(256))) @ v           -> [64,512,256]

Sharding: 8 NeuronCores, 8 batches per core (pure data parallel, weights
replicated, no collectives). Each core runs the same program on its shard.

v3 design:
  * All matmul operands bf16 (1 cycle/row on PE at any width; rel err ~3e-3
    vs the fp32 reference, well under the 2e-2 gate).
  * x pre-transposed + pre-cast ON HOST to xT[b, d_model, T] bf16 -- zero
    PE transposes for x.
  * Scores computed TRANSPOSED: sT[tk, tq] strips (lhsT=kT chunk, rhs=qT),
    so the exp'd weights leave the ACT engine already in the [tk, tq]
    layout the AV matmul wants as stationary -- no weight transposes.
    Softmax denominator comes free as a ones-column appended to v (AV out
    col 256 accumulates l = sum_tk exp per query row).
  * bk dropped (constant-per-query shift cancels in softmax); bq
    pre-scaled by 1/sqrt(d) on host; bv folded into the v drain
    (out = sum w (v+bv) / l == out + bv exactly, since l = sum w).
  * Weight/bias loads hoisted out of the reps loop (loop-invariant).
  * reps>1 timing builds use a rotated software pipeline across the For_i
    boundary: prologue emits prep(pair0); the body riffles attn(p_i) with
    prep(p_{i+1 mod 4}).  The trailing prep(p0') lands on the same ring
    slots the body-head attn(p0) reads (ring sizes divide the per-iteration
    allocation count), so iteration n+1's attention correctly consumes
    iteration n's trailing loads -- the PE never drains at the loop seam.
    Work per iteration is identical to the reps=1 build.
  * DMA: x loads alternate the two HWDGE queues (SP + ACT); out stores ride
    the gpsimd SWDGE queues (measured faster than sharing the HWDGE rings).

Per-batch PE work: 16384 (q/k proj) + 8192 (v proj) + 2560 (scores)
+ 2570 (AV) = 29706 cycles ~= 12.4us @2.4GHz; 8 batches/core ~= 99us.
"""

import numpy as np

import concourse.bass as bass
import concourse.mybir as mybir
import concourse.tile as tile
from concourse import bacc
from concourse.bass_utils import run_bass_kernel_spmd

B, T, DM, D = 64, 512, 1024, 256
NCORES = 8
BPC = B // NCORES  # batches per core
P = 128
KO = DM // P  # 8 contraction subtiles for the projections
NCH = T // P  # 4 token chunks per sequence
DJ = D // P  # 2 head-dim chunks
SCALE = 1.0 / 16.0  # 256 ** -0.5
MASK_VAL = -1e30
DW = D + 1  # AV moving width: 256 v columns + 1 ones column (row-sum)

F32 = mybir.dt.float32
BF16 = mybir.dt.bfloat16

# score strip s covers queries tq in [s*128, 512): widths and packed offsets
SWID = [T - s * P for s in range(NCH)]  # 512, 384, 256, 128
SOFF = [0, 512, 896, 1152]
WTOT = 1280


def emit_core_program(ctx, nc: bass.Bass, tc, io, reps=1, hints=True,
                      staggered=False, rotate=True, out_gp=True, x_split=True,
                      pqk_bufs=3, ps_bufs=2, pav_bufs=3, xt_bufs=4,
                      out_dve=False, mask_gp=False, split_exp=False,
                      vt_proj=False, vdma=False):
    xt_d, wq_d, bq_d, wk_d, wv_d, bv_d, out_d = io

    def enter_pool(name, bufs, space="SBUF"):
        return ctx.enter_context(tc.tile_pool(name=name, bufs=bufs, space=space))

    consts = enter_pool("consts", bufs=1)
    # transposed causal additive mask for the diagonal block of each strip:
    # keep (0) where tq_local >= tk_local i.e. col >= row, else -1e30
    cmask = consts.tile([P, P], F32, name="cmask")
    nc.gpsimd.memset(cmask, 0.0)
    nc.gpsimd.affine_select(
        out=cmask, in_=cmask, compare_op=mybir.AluOpType.is_ge,
        fill=MASK_VAL, base=0, pattern=[[1, P]], channel_multiplier=-1,
    )

    if vdma:
        vt_proj = False
    if vt_proj:
        identf = consts.tile([P, P], F32, name="identf")
        from concourse.masks import make_identity
        make_identity(nc, identf)
        identr = consts.tile([P, P], BF16, name="identr")
        nc.vector.tensor_copy(identr, identf)

    wq_s = consts.tile([P, KO, D], BF16, name="wq_s")
    wk_s = consts.tile([P, KO, D], BF16, name="wk_s")
    wv_s = consts.tile([P, KO, D], BF16, name="wv_s")
    bq_s = consts.tile([P, DJ], F32, name="bq_s")  # host pre-scaled by 1/16
    bv_s = consts.tile([P, D], F32, name="bv_s")
    bv16_s = consts.tile([P, D], BF16, name="bv16_s") if vdma else None

    # one-time loads (outside the reps loop: loop-invariant)
    nc.scalar.dma_start(wq_s, wq_d.rearrange("(ko p) d -> p ko d", p=P))
    nc.scalar.dma_start(wk_s, wk_d.rearrange("(ko p) d -> p ko d", p=P))
    nc.scalar.dma_start(wv_s, wv_d.rearrange("(ko p) d -> p ko d", p=P))
    nc.gpsimd.dma_start(bq_s, bq_d.rearrange("(j p) -> p j", p=P))
    nc.gpsimd.dma_start(bv_s, bv_d[None, :].to_broadcast((P, D)))
    if vdma:
        nc.vector.tensor_copy(bv16_s, bv_s)

    xt_pool = enter_pool("xt", bufs=xt_bufs)
    qk_pool = enter_pool("qk", bufs=4 if vdma else 2)
    v_pool = enter_pool("v", bufs=4 if vdma else 2)
    wt_pool = enter_pool("wt", bufs=2)
    o_pool = enter_pool("o", bufs=4)
    g_pool = enter_pool("g", bufs=2) if vdma else None
    stat_pool = enter_pool("stat", bufs=8)
    ps_qk = enter_pool("ps_qk", bufs=pqk_bufs, space="PSUM")
    ps_s = enter_pool("ps_s", bufs=ps_bufs, space="PSUM")
    ps_av = enter_pool("ps_av", bufs=pav_bufs, space="PSUM")

    def load_stages(b, split=False):
        """x^T DMA for one batch, two chunks for pipelining granularity.
        x_split (or startup split=True): chunks alternate the two HWDGE
        queues (SP + ACT) so two HBM streams run in parallel."""
        xt = xt_pool.tile([P, KO, T], BF16, name="xt", tag="xt")
        xr = xt_d[b].rearrange("(ko p) t -> p ko t", p=P)
        h = KO // 2

        def dma_lo():
            # vdma: keep the HWDGE rings exclusively for the v transposes
            eng = nc.gpsimd if vdma else (
                nc.sync if (split or x_split) else nc.scalar)
            eng.dma_start(xt[:, :h, :], xr[:, :h, :])

        def dma_hi():
            eng = nc.gpsimd if vdma else nc.scalar
            eng.dma_start(xt[:, h:, :], xr[:, h:, :])

        return xt, [dma_lo, dma_hi]

    def qk_proj_stages(xts):
        """Emit-closures, one per (proj, j): 16 paired matmuls + drains.
        The stationary weight chunk feeds both batches' moving operands.
        With vt_proj, the v projection also runs here Wv-stationary
        (512-wide moving, 4x fewer LDWEIGHTS), producing vT to be
        PE-transposed in the attention stages."""
        labels = ("q", "k") + (("v",) if (vt_proj or vdma) else ())
        ws = {"q": wq_s, "k": wk_s, "v": wv_s}
        dsts = {}
        for lbl in labels:
            dsts[lbl] = [
                qk_pool.tile([P, DJ, T], BF16, name="qkt", tag=f"qkt{i}{lbl}")
                for i in range(len(xts))
            ]

        def group(lbl, j):
            w_s = ws[lbl]
            pms = [ps_qk.tile([P, T], F32, name="pm", tag="pqk") for _ in xts]
            for ko in range(KO):
                for i, xt in enumerate(xts):
                    nc.tensor.matmul(
                        pms[i],
                        w_s[:, ko, j * P:(j + 1) * P],
                        xt[:, ko, :],
                        start=(ko == 0),
                        stop=(ko == KO - 1),
                    )
            for i in range(len(xts)):
                if lbl == "q":
                    # qT = psum*1/16 + bq/16, drained on ACT (per-part bias)
                    nc.scalar.activation(
                        dsts["q"][i][:, j, :], pms[i],
                        mybir.ActivationFunctionType.Identity,
                        bias=bq_s[:, j:j + 1], scale=SCALE,
                    )
                else:
                    nc.vector.tensor_copy(dsts[lbl][i][:, j, :], pms[i])

        return (tuple(dsts[lbl] for lbl in labels),
                [lambda lbl=lbl, j=j: group(lbl, j)
                 for lbl in labels for j in range(DJ)])

    def make_prep(b0, split=False):
        xt0, ls0 = load_stages(b0, split)
        xt1, ls1 = load_stages(b0 + 1, split)
        prep = [s for pair in zip(ls0, ls1) for s in pair]
        dsts, qs = qk_proj_stages([xt0, xt1])
        return (xt0, xt1) + dsts, prep + qs

    def attention_stages(b, xt, qt, kt, vt=None):
        """Schedulable closures for one batch: 4 v-projection chunks,
        then S (scores strip) / V (AV chunk) stages."""
        vw = 384 if vdma else DW + 7  # vdma: 256B-aligned chunk offsets
        v_sb = v_pool.tile([P, NCH, vw], BF16, name="v_sb", tag=f"v{b % 2}")
        wt = wt_pool.tile([P, WTOT], BF16, name="wt", tag=f"wt{b % 2}")

        gate = {}

        def v_tr(h):
            # SBUF->SBUF DMA transposes: vT[dv_j, tok_c] -> v[tok_c, dv_j].
            # All of one batch's transposes ride ONE serial HWDGE ring; the
            # trailing bv-copy (tracked InstDMACopy) on the same ring then
            # completes strictly after them, and v_add consuming that copy
            # gives the DMA->engine ordering the tracker fails to emit for
            # InstDmaTransposeAnt itself.
            if h == 0:
                nc.gpsimd.memset(v_sb[:, :, D:D + 1], 1.0)  # ones col -> l
            eng = nc.sync if b % 2 else nc.scalar
            for c in (2 * h, 2 * h + 1):
                for j in range(DJ):
                    eng.dma_start(v_sb[:, c, j * P:(j + 1) * P],
                                  vt[:, j, c * P:(c + 1) * P], transpose=True)
            if h == 1:
                bvg = g_pool.tile([P, D], BF16, name="bvg", tag=f"g{b % 2}")
                eng.dma_start(bvg, bv16_s)
                gate["bv"] = bvg

        def v_add(h):
            # v + bv fused (exact through softmax since rows sum to 1)
            for c in (2 * h, 2 * h + 1):
                nc.vector.tensor_add(v_sb[:, c, :D], v_sb[:, c, :D],
                                     gate["bv"])

        def v_chunk(c):
            if c == 0:
                nc.gpsimd.memset(v_sb[:, :, D:D + 1], 1.0)  # ones col -> l
            if vt_proj:
                # vT already computed (Wv-stationary); transpose back chunkwise
                pv = ps_av.tile([P, D], BF16, name="pv", tag="pvt")
                for j in range(DJ):
                    nc.tensor.transpose(
                        pv[:, j * P:(j + 1) * P],
                        vt[:, j, c * P:(c + 1) * P], identr,
                    )
            else:
                pv = ps_av.tile([P, DW + 7], F32, name="pv", tag="pav")
                for ko in range(KO):
                    nc.tensor.matmul(
                        pv[:, :D],
                        xt[:, ko, c * P:(c + 1) * P],
                        wv_s[:, ko, :],
                        start=(ko == 0),
                        stop=(ko == KO - 1),
                    )
            # v + bv fused into the drain: out = sum w (v+bv) / l == out + bv
            nc.vector.tensor_add(v_sb[:, c, :D], pv[:, :D], bv_s)

        def stage_s(s):
            wid = SWID[s]
            ps = ps_s.tile([P, T], F32, name="ps", tag="ps")
            for j in range(DJ):
                nc.tensor.matmul(
                    ps[:, :wid],
                    kt[:, j, s * P:(s + 1) * P],
                    qt[:, j, s * P:],
                    start=(j == 0),
                    stop=(j == DJ - 1),
                )
            # additive causal mask on the diagonal (first) block of the strip
            Exp = mybir.ActivationFunctionType.Exp
            if split_exp and wid > P:
                # off-diagonal exp has no mask dependency: issue it first
                nc.scalar.activation(
                    wt[:, SOFF[s] + P:SOFF[s] + wid], ps[:, P:wid], Exp,
                )
            madd = nc.gpsimd.tensor_add if mask_gp else nc.vector.tensor_add
            madd(ps[:, :P], ps[:, :P], cmask)
            # scores are O(1): exp without max-subtraction; masked -> exp=0
            if split_exp:
                nc.scalar.activation(wt[:, SOFF[s]:SOFF[s] + P], ps[:, :P], Exp)
            else:
                nc.scalar.activation(
                    wt[:, SOFF[s]:SOFF[s] + wid], ps[:, :wid], Exp,
                )

        def stage_v(c):
            po = ps_av.tile([P, DW + 7], F32, name="po", tag="pav")
            for s in range(c + 1):
                off = SOFF[s] + (c - s) * P
                nc.tensor.matmul(
                    po[:, :DW], wt[:, off:off + P], v_sb[:, s, :DW],
                    start=(s == 0), stop=(s == c),
                )
            linv = stat_pool.tile([P, 1], F32, name="linv", tag="linv")
            nc.vector.reciprocal(linv, po[:, D:D + 1])
            ot = o_pool.tile([P, D], F32, name="ot", tag="ot")
            if out_dve:
                nc.vector.tensor_scalar_mul(ot, po[:, :D], linv)
            else:
                nc.scalar.activation(
                    ot, po[:, :D], mybir.ActivationFunctionType.Copy,
                    scale=linv,
                )
            eng = nc.gpsimd if out_gp else nc.sync
            eng.dma_start(out_d[b, c * P:(c + 1) * P, :], ot)

        if vdma:
            stages = [("vp", v_tr, 0), ("vp", v_tr, 1),
                      ("vp", v_add, 0), ("vp", v_add, 1)]
        else:
            stages = [("vp", v_chunk, c) for c in range(NCH)]
        order = [("s", 0), ("s", 1), ("v", 0), ("s", 2), ("v", 1),
                 ("s", 3), ("v", 2), ("v", 3)]
        fmap = {"s": stage_s, "v": stage_v}
        stages += [(kk, fmap[kk], c) for kk, c in order]
        return stages

    def make_attn(b0, pctx):
        xt0, xt1, qts, kts = pctx[:4]
        vts = pctx[4] if (vt_proj or vdma) else (None, None)
        a0 = attention_stages(b0, xt0, qts[0], kts[0], vts[0])
        a1 = attention_stages(b0 + 1, xt1, qts[1], kts[1], vts[1])
        return [s for pair in zip(a0, a1) for s in pair]

    def riffle_run(attn, prep):
        n = max(len(attn), len(prep))
        for i in range(n):
            if i < len(attn):
                _k, fn, c = attn[i]
                fn(c)
            if i < len(prep):
                prep[i]()

    pairs = list(range(0, BPC, 2))

    he = (
        mybir.EngineType.PE, mybir.EngineType.DVE,
        mybir.EngineType.Activation, mybir.EngineType.SP,
    ) if hints else ()

    if reps > 1 and rotate:
        # software-pipelined across the loop seam: prologue prep, body is
        # attn(p_i) riffled with prep(p_{i+1 mod 4}); the trailing prep
        # lands on the same ring slots the body-head attn reads.
        pctx, prep = make_prep(pairs[0], split=True)
        for s in prep:
            s()
        ctx.enter_context(tc.For_i(0, reps, 1, hint_engines=he,
                                   staggered_reset=staggered))
        for i, b0 in enumerate(pairs):
            attn = make_attn(b0, pctx)
            nxt = pairs[(i + 1) % len(pairs)]
            pctx, prep = make_prep(nxt)
            riffle_run(attn, prep)
        return

    if reps > 1:
        ctx.enter_context(tc.For_i(0, reps, 1, hint_engines=he,
                                   staggered_reset=staggered))

    pending = None
    for i, b0 in enumerate(pairs):
        pctx, prep = make_prep(b0, split=(i == 0))
        if pending is None:
            for s in prep:
                s()
        else:
            riffle_run(pending, prep)
        pending = make_attn(b0, pctx)
    riffle_run(pending, [])


def build_program(reps=1, hints=True, **flags):
    """Build the single-core Bass program (same program runs on all 8 cores).

    reps > 1 wraps the whole body in a hardware loop (same work each
    iteration) -- used only for device-time measurement."""
    nc = bacc.Bacc("TRN2", target_bir_lowering=False, debug=False)
    xt_d = nc.dram_tensor("x", [BPC, DM, T], BF16, kind="ExternalInput").ap()
    wq_d = nc.dram_tensor("wq", [DM, D], BF16, kind="ExternalInput").ap()
    bq_d = nc.dram_tensor("bq", [D], F32, kind="ExternalInput").ap()
    wk_d = nc.dram_tensor("wk", [DM, D], BF16, kind="ExternalInput").ap()
    wv_d = nc.dram_tensor("wv", [DM, D], BF16, kind="ExternalInput").ap()
    bv_d = nc.dram_tensor("bv", [D], F32, kind="ExternalInput").ap()
    out_d = nc.dram_tensor("out", [BPC, T, D], F32, kind="ExternalOutput").ap()

    from contextlib import ExitStack

    with tile.TileContext(nc) as tc, ExitStack() as ctx:
        emit_core_program(
            ctx, nc, tc, (xt_d, wq_d, bq_d, wk_d, wv_d, bv_d, out_d),
            reps=reps, hints=hints, **flags,
        )
    nc.compile()
    return nc


_NC_CACHE = None


def _get_program():
    global _NC_CACHE
    if _NC_CACHE is None:
        _NC_CACHE = build_program()
    return _NC_CACHE


def make_in_maps(inputs):
    import ml_dtypes
    bf16 = ml_dtypes.bfloat16
    x = np.asarray(inputs["x"], dtype=np.float32)
    # host-side: transpose to [B, d_model, T] and cast to bf16
    xt = np.ascontiguousarray(x.transpose(0, 2, 1)).astype(bf16)
    shared = {
        "wq": np.ascontiguousarray(np.asarray(inputs["Wq"], np.float32)).astype(bf16),
        "bq": np.ascontiguousarray(
            np.asarray(inputs["bq"], np.float32) * np.float32(SCALE)),
        "wk": np.ascontiguousarray(np.asarray(inputs["Wk"], np.float32)).astype(bf16),
        "wv": np.ascontiguousarray(np.asarray(inputs["Wv"], np.float32)).astype(bf16),
        "bv": np.ascontiguousarray(np.asarray(inputs["bv"], np.float32)),
    }
    return [
        {"x": xt[i * BPC:(i + 1) * BPC], **shared} for i in range(NCORES)
    ]


def kernel(**inputs) -> np.ndarray:
    nc = _get_program()
    in_maps = make_in_maps(inputs)
    res = run_bass_kernel_spmd(nc, in_maps, core_ids=list(range(NCORES)))
    return np.concatenate([m["out"] for m in res.results], axis=0)
